# revision 1
# baseline (speedup 1.0000x reference)
"""Self-contained Trainium2 Bass kernel for nn_AutoRegressive_88837103551116.

2-layer LSTM (HID=64) over ragged sequences: warmup pass over x (per-sample
lengths), then autoregressive decode over [dense(h_top_final), context_t].
Pure data-parallel over 8 NeuronCores (batch 512 -> 64 per core).

Device algorithm (per core):
  - slot s computes layer0 @ step s and layer1 @ step s-1 (layer stagger) so
    both layers' gates share each tick's instructions
  - states [feature, batch]: rb [128,B]=[h0;h1], cc [64,2B]=[c0|c1]
  - gates via 8 small matmuls/tick into 2 PSUM banks (one accumulation group
    open per bank at a time; recurrence-independent matmuls lead each tick so
    the PE runs ahead); biases and the ragged-sequence c-freeze (+/-BIG added
    to i/f pre-activations past each sample's length) are folded into extra
    stationary-weight rows
  - h at the last valid step is captured into hkeep via copy_predicated with
    uint8 mask streams (off the recurrence critical path)
  - host side: input transposes/stream building, output -999 masking
"""
import sys

import numpy as np

try:
    import concourse.bass as bass
except ImportError:
    sys.path.insert(0, "/opt/trn_rl_repo")
    import concourse.bass as bass

import contextlib
import json

import concourse.tile as tile
from concourse import mybir
from concourse.bass_utils import run_bass_kernel_spmd

N_CORES = 8
TW = 512
TC = 512



H = 64
IN = 16
F = 8
C = 8
BIG = 50.0


def build_weights(Wih0, Whh0, bih0, bhh0, Wih1, Whh1, bih1, bhh1, Wd, bd):
    """Build all stationary lhsT matrices (shared across cores). fp32."""
    b0 = bih0 + bhh0
    b1 = bih1 + bhh1
    maskcol_if = np.concatenate([np.full(H, -BIG), np.full(H, BIG)]).astype(np.float32)

    def stack_l0(gate_rows, xw, bias, mask):
        # lhsT [18, 128]: rows 0:16 = xw^T, 16 = mask, 17 = bias
        out = np.zeros((18, 128), np.float32)
        out[0:xw.shape[1], :] = xw[gate_rows].T
        out[16] = mask
        out[17] = bias[gate_rows]
        return out

    gi = slice(0, 128)   # i,f rows
    gg = slice(128, 256)  # g,o rows
    W = {}
    W["w0x_if"] = stack_l0(gi, Wih0, b0, maskcol_if)
    W["w0x_go"] = stack_l0(gg, Wih0, b0, np.zeros(128, np.float32))
    W["w0h_if"] = Whh0[gi].T.copy()   # [64,128]
    W["w0h_go"] = Whh0[gg].T.copy()
    W["w1_if"] = np.concatenate([Wih1[gi].T, Whh1[gi].T], 0)  # [128,128]
    W["w1_go"] = np.concatenate([Wih1[gg].T, Whh1[gg].T], 0)
    W["wc_if"] = np.stack([b1[gi], maskcol_if]).astype(np.float32)  # [2,128]
    W["wc_go"] = b1[gg].reshape(1, 128).astype(np.float32)
    # decode l0: input rows 0:8 = ctx weights (cols 8:16 of Wih0), elem separate
    def stack_l0_dec(gate_rows, mask):
        out = np.zeros((18, 128), np.float32)
        out[0:8, :] = Wih0[gate_rows, 8:16].T
        out[16] = mask
        out[17] = b0[gate_rows]
        return out
    W["d0x_if"] = stack_l0_dec(gi, maskcol_if)
    W["d0x_go"] = stack_l0_dec(gg, np.zeros(128, np.float32))
    W["d0e_if"] = Wih0[gi, 0:8].T.copy()  # [8,128]
    W["d0e_go"] = Wih0[gg, 0:8].T.copy()
    W["wdT"] = Wd.T.copy()  # [64,8]
    W["bd"] = bd.reshape(8, 1).copy()
    for k in W:
        W[k] = np.ascontiguousarray(W[k], np.float32)
    return W


def build_streams(x, lengths_x, context, lengths_ctx):
    """Per-core streams. x [B,TW,16], context [B,TC,8]. B arbitrary."""
    B = x.shape[0]
    TW = x.shape[1]
    TC = context.shape[1]
    SW, SD = 528, 512
    CW, CD = SW // 8, SD // 8

    s_idx = np.arange(SW)
    mw = (s_idx[:, None] < lengths_x[None, :]).astype(np.float32)  # [SW,B]
    mw1 = np.zeros_like(mw)
    mw1[1:] = mw[:-1]

    WA = np.zeros((CW, 18, 8, B), np.float32)
    xt = np.transpose(x, (1, 2, 0))  # [TW,16,B]
    WA[:, 0:16].reshape(CW * 8 * 16, B)
    WA_r = WA.reshape(CW, 18, 8, B)
    for c in range(CW):
        for t in range(8):
            s = c * 8 + t
            if s < TW:
                WA_r[c, 0:16, t, :] = xt[s]
            WA_r[c, 16, t, :] = 1.0 - mw[s]
            WA_r[c, 17, t, :] = 1.0
    WC = np.zeros((CW, 2, 8, B), np.float32)
    WC[:, 0] = 1.0
    WC[:, 1] = (1.0 - mw1).reshape(CW, 8, B)
    NMw = np.zeros((CW, 128, 8, B), np.uint8)
    NMw[:, 0:64] = mw.reshape(CW, 8, 1, B).transpose(0, 2, 1, 3)
    NMw[:, 64:128] = mw1.reshape(CW, 8, 1, B).transpose(0, 2, 1, 3)

    md0 = np.zeros((SD, B), np.float32)
    md0[0:TC - 1] = 1.0     # l0 steps 0..510 active; 511 pad frozen
    md1 = np.ones((SD, B), np.float32)
    md1[0] = 0.0            # freeze l1 at slot 0
    DA = np.zeros((CD, 18, 8, B), np.float32)
    ctxt = np.transpose(context, (1, 2, 0))  # [TC,8,B]
    DA_r = DA
    for c in range(CD):
        for t in range(8):
            s = c * 8 + t
            if s < TC - 1:
                DA_r[c, 0:8, t, :] = ctxt[s]
            DA_r[c, 16, t, :] = 1.0 - md0[s]
            DA_r[c, 17, t, :] = 1.0
    DC = np.zeros((CD, 2, 8, B), np.float32)
    DC[:, 0] = 1.0
    DC[:, 1] = (1.0 - md1).reshape(CD, 8, B)
    NMd = np.zeros((CD, 128, 8, B), np.uint8)
    NMd[:, 0:64] = (1.0 - md0).reshape(CD, 8, 1, B).transpose(0, 2, 1, 3)
    NMd[:, 64:128] = (1.0 - md1).reshape(CD, 8, 1, B).transpose(0, 2, 1, 3)

    def pad1(a):
        return np.concatenate([a, np.zeros_like(a[:1])], 0)
    return dict(
        wa=pad1(WA.reshape(CW, 18, 8 * B)),
        wcs=pad1(WC.reshape(CW, 2, 8 * B)),
        nmw=pad1(NMw.reshape(CW, 128, 8 * B)),
        da=pad1(DA.reshape(CD, 18, 8 * B)),
        dcs=pad1(DC.reshape(CD, 2, 8 * B)),
        nmd=pad1(NMd.reshape(CD, 128, 8 * B)),
    )


def post_outputs(YE, YD, bd, lengths_ctx, TC):
    """YE [8,B], YD [CD,8,8,B] -> out [B,TC,8] with -999 padding."""
    B = YE.shape[1]
    out = np.zeros((B, TC, F), np.float32)
    out[:, 0, :] = YE.T
    ysd = YD.transpose(0, 2, 1, 3).reshape(512, F, B)  # [slot, F, B]
    # ys_t = slot t+1 for t = 0..510
    out[:, 1:TC, :] = ysd[1:TC].transpose(2, 0, 1) + bd[None, None, :]
    valid = np.arange(TC)[None, :] < lengths_ctx[:, None]
    return np.where(valid[:, :, None], out, np.float32(-999.0))



import contextlib

import concourse.bass as bass
import concourse.tile as tile
from concourse import mybir

F32 = mybir.dt.float32
U8 = mybir.dt.uint8
AF = mybir.ActivationFunctionType

B = 64
H = 64
SW = 528   # warmup slots (padded; needs >= 513)
SD = 512   # decode slots (l0 steps 0..510 + 1 pad)
CW = SW // 8
CD = SD // 8


def build_nc(repeat=1, static=False):
    nc = bass.Bass("TRN2", target_bir_lowering=False, debug=False)
    dt = F32

    d = {}
    d["wa"] = nc.dram_tensor("wa", [CW + 1, 18, 512], dt, kind="ExternalInput")
    d["wcs"] = nc.dram_tensor("wcs", [CW + 1, 2, 512], dt, kind="ExternalInput")
    d["nmw"] = nc.dram_tensor("nmw", [CW + 1, 128, 512], U8, kind="ExternalInput")
    d["da"] = nc.dram_tensor("da", [CD + 1, 18, 512], dt, kind="ExternalInput")
    d["dcs"] = nc.dram_tensor("dcs", [CD + 1, 2, 512], dt, kind="ExternalInput")
    for name, shp in [
        ("w0x_if", [18, 128]), ("w0x_go", [18, 128]),
        ("w0h_if", [64, 128]), ("w0h_go", [64, 128]),
        ("w1_if", [128, 128]), ("w1_go", [128, 128]),
        ("wc_if", [2, 128]), ("wc_go", [1, 128]),
        ("d0x_if", [18, 128]), ("d0x_go", [18, 128]),
        ("d0e_if", [8, 128]), ("d0e_go", [8, 128]),
        ("wdT", [64, 8]), ("bd", [8, 1]),
    ]:
        d[name] = nc.dram_tensor(name, shp, dt, kind="ExternalInput")
    ye = nc.dram_tensor("ye", [8, B], dt, kind="ExternalOutput")
    yd = nc.dram_tensor("yd", [CD, 8, 512], dt, kind="ExternalOutput")

    with tile.TileContext(nc) as tc:
        with (
            tc.tile_pool(name="consts", bufs=1) as consts,
            tc.tile_pool(name="state", bufs=1) as state,
            tc.tile_pool(name="stream", bufs=1) as stream,
            tc.tile_pool(name="work", bufs=3) as work,
            tc.tile_pool(name="psum", bufs=2, space="PSUM") as psum,
            tc.tile_pool(name="outp", bufs=1, space="PSUM") as outp,
        ):
            W = {}
            for name in ["w0x_if", "w0x_go", "w0h_if", "w0h_go", "w1_if",
                         "w1_go", "wc_if", "wc_go", "d0x_if", "d0x_go",
                         "d0e_if", "d0e_go", "bd"]:
                t = consts.tile(list(d[name].shape), dt, tag=name, name="w_" + name)
                nc.sync.dma_start(out=t, in_=d[name][:, :])
                W[name] = t
            wdT_t = consts.tile([128, 8], dt, tag="wdT", name="w_wdT")
            nc.sync.dma_start(out=wdT_t[64:128, :], in_=d["wdT"][:, :])
            W["wdT"] = wdT_t

            rb = [state.tile([128, B], dt, tag=f"rb{i}", name=f"rb{i}") for i in range(2)]
            cc = [state.tile([64, 2 * B], dt, tag=f"cc{i}", name=f"cc{i}") for i in range(2)]
            for i in range(2):
                nc.vector.memset(rb[i], 0.0)
                nc.vector.memset(cc[i], 0.0)

            saA = stream.tile([18, 512], dt, tag="saA")
            saB = stream.tile([18, 512], dt, tag="saB")
            scA = stream.tile([2, 512], dt, tag="scA")
            scB = stream.tile([2, 512], dt, tag="scB")
            nmA = stream.tile([128, 512], U8, tag="nmA")
            nmB = stream.tile([128, 512], U8, tag="nmB")
            elem = state.tile([8, B], dt, tag="elem")
            hkeep = state.tile([128, B], dt, tag="hkeep")
            nc.vector.memset(hkeep, 0.0)

            def tick(sl, sa, sc, nm, decode):
                par = sl % 2
                rbp, rbn = rb[par], rb[1 - par]
                ccp, ccn_dst = cc[par], cc[1 - par]
                t8 = sl % 8
                colB = slice(t8 * B, (t8 + 1) * B)
                megaIF = psum.tile([128, 2 * B], dt, tag="megaIF", name="megaIF")
                megaGO = psum.tile([128, 2 * B], dt, tag="megaGO", name="megaGO")

                wx_if = W["d0x_if"] if decode else W["w0x_if"]
                wx_go = W["d0x_go"] if decode else W["w0x_go"]

                # Gate matmuls. Two PSUM banks (IF / GO), at most one open
                # accumulation group per bank; the rbp-independent leading MMs
                # let the PE run ahead during the previous tick's tail.
                if decode:
                    nc.tensor.matmul(megaIF[:, 0:B], W["d0e_if"], elem, start=True, stop=False)
                    nc.tensor.matmul(megaGO[:, 0:B], W["d0e_go"], elem, start=True, stop=False)
                    nc.tensor.matmul(megaIF[:, 0:B], wx_if, sa[0:18, colB], start=False, stop=False)
                    nc.tensor.matmul(megaGO[:, 0:B], wx_go, sa[0:18, colB], start=False, stop=False)
                else:
                    nc.tensor.matmul(megaIF[:, 0:B], wx_if, sa[0:18, colB], start=True, stop=False)
                    nc.tensor.matmul(megaGO[:, 0:B], wx_go, sa[0:18, colB], start=True, stop=False)
                nc.tensor.matmul(megaIF[:, 0:B], W["w0h_if"], rbp[0:64, :], start=False, stop=True)
                nc.tensor.matmul(megaIF[:, B:2 * B], W["wc_if"], sc[0:2, colB], start=True, stop=False)
                nc.tensor.matmul(megaIF[:, B:2 * B], W["w1_if"], rbp[:, :], start=False, stop=True)
                nc.tensor.matmul(megaGO[:, 0:B], W["w0h_go"], rbp[0:64, :], start=False, stop=True)
                nc.tensor.matmul(megaGO[:, B:2 * B], W["wc_go"], sc[0:1, colB], start=True, stop=False)
                nc.tensor.matmul(megaGO[:, B:2 * B], W["w1_go"], rbp[:, :], start=False, stop=True)

                # activations (sif in PSUM: exempts t1/t2 from equal-base rule)
                sif = psum.tile([128, 2 * B], dt, tag="sif", name="sif")
                tg = work.tile([64, 2 * B], dt, tag="tg", name="tg")
                so = work.tile([64, 2 * B], dt, tag="so", name="so")
                nc.scalar.activation(sif, megaIF[:, :], AF.Sigmoid)
                nc.scalar.activation(tg, megaGO[0:64, :], AF.Tanh)
                nc.scalar.activation(so, megaGO[64:128, :], AF.Sigmoid)

                # elementwise
                t1 = work.tile([64, 2 * B], dt, tag="t1", name="t1")
                t2 = work.tile([64, 2 * B], dt, tag="t2", name="t2")
                th = work.tile([64, 2 * B], dt, tag="th", name="th")
                nc.vector.tensor_mul(t1, sif[0:64, :], tg)
                nc.vector.tensor_mul(t2, sif[64:128, :], ccp)
                nc.vector.tensor_add(ccn_dst, t1, t2)
                nc.scalar.activation(th, ccn_dst, AF.Tanh)
                nc.vector.tensor_mul(rbn[0:64, :], so[:, 0:B], th[:, 0:B])
                nc.gpsimd.tensor_mul(rbn[64:128, :], so[:, B:2 * B], th[:, B:2 * B])

                if nm is not None:
                    # capture h at each sample's last active slot (off the
                    # recurrence critical path)
                    nc.vector.copy_predicated(hkeep, nm[:, colB], rbn)
                return rbn

            def loop(n):
                # static: python-unrolled; else: hardware For_i
                if static:
                    return contextlib.nullcontext(enumerate(range(n)))
                return None

            rep_cm = tc.For_i(0, repeat, 1) if repeat > 1 else contextlib.nullcontext()
            with rep_cm:
                # ================= warmup =================
                nc.sync.dma_start(out=saA, in_=d["wa"][0, :, :])
                nc.sync.dma_start(out=scA, in_=d["wcs"][0, :, :])
                nc.sync.dma_start(out=nmA, in_=d["nmw"][0, :, :])
                def warm_body(j, i1, i2, first=False):
                    nc.sync.dma_start(out=saB, in_=d["wa"][i1, :, :])
                    nc.sync.dma_start(out=scB, in_=d["wcs"][i1, :, :])
                    nc.sync.dma_start(out=nmB, in_=d["nmw"][i1, :, :])
                    for sl in range(8):
                        tick(sl, saA, scA, nmA, False)
                        if first and sl == 0:
                            nc.vector.memset(rb[1][64:128, :], 0.0)
                    nc.sync.dma_start(out=saA, in_=d["wa"][i2, :, :])
                    nc.sync.dma_start(out=scA, in_=d["wcs"][i2, :, :])
                    nc.sync.dma_start(out=nmA, in_=d["nmw"][i2, :, :])
                    for sl in range(8, 16):
                        tick(sl, saB, scB, nmB, False)

                if static:
                    for j in range(CW // 2):
                        warm_body(j, j * 2 + 1, j * 2 + 2, first=(j == 0))
                else:
                    warm_body(0, 1, 2, first=True)
                    with tc.For_i(1, CW // 2, 1, hint_engines=(mybir.EngineType.PE,)) as j:
                        warm_body(j, nc.snap(j * 2 + 1), nc.snap(j * 2 + 2))

                # ================= elem =================
                nc.vector.tensor_copy(rb[0], hkeep)
                pe = outp.tile([8, B], dt, tag="ops", name="pe")
                nc.tensor.matmul(pe, W["wdT"][64:128, :], rb[0][64:128, :], start=True, stop=True)
                nc.scalar.activation(elem, pe, AF.Identity, bias=W["bd"][:, 0:1])
                nc.sync.dma_start(out=ye[:, :], in_=elem)

                # ================= decode =================
                nc.sync.dma_start(out=saA, in_=d["da"][0, :, :])
                nc.sync.dma_start(out=scA, in_=d["dcs"][0, :, :])
                def dec_body(j, i0, i1, i2, first=False):
                    nc.sync.dma_start(out=saB, in_=d["da"][i1, :, :])
                    nc.sync.dma_start(out=scB, in_=d["dcs"][i1, :, :])
                    ops = outp.tile([8, 512], dt, tag="ops", name="ops")
                    for sl in range(8):
                        rbn = tick(sl, saA, scA, None, True)
                        if first and sl == 0:
                            nc.vector.tensor_copy(rb[1][64:128, :], rb[0][64:128, :])
                        nc.tensor.matmul(ops[:, sl * B:(sl + 1) * B], W["wdT"][64:128, :],
                                         rbn[64:128, :], start=True, stop=True)
                    oso = work.tile([8, 512], dt, tag="oso", name="oso")
                    nc.scalar.copy(oso, ops)
                    nc.sync.dma_start(out=yd[i0, :, :], in_=oso)
                    nc.sync.dma_start(out=saA, in_=d["da"][i2, :, :])
                    nc.sync.dma_start(out=scA, in_=d["dcs"][i2, :, :])
                    ops2 = outp.tile([8, 512], dt, tag="ops2", name="ops2")
                    for sl in range(8, 16):
                        rbn = tick(sl, saB, scB, None, True)
                        nc.tensor.matmul(ops2[:, (sl - 8) * B:(sl - 7) * B], W["wdT"][64:128, :],
                                         rbn[64:128, :], start=True, stop=True)
                    oso2 = work.tile([8, 512], dt, tag="oso2", name="oso2")
                    nc.scalar.copy(oso2, ops2)
                    nc.sync.dma_start(out=yd[i1, :, :], in_=oso2)

                if static:
                    for j in range(CD // 2):
                        dec_body(j, j * 2, j * 2 + 1, j * 2 + 2, first=(j == 0))
                else:
                    dec_body(0, 0, 1, 2, first=True)
                    with tc.For_i(1, CD // 2, 1, hint_engines=(mybir.EngineType.PE,)) as j:
                        dec_body(j, nc.snap(j * 2), nc.snap(j * 2 + 1), nc.snap(j * 2 + 2))

    return nc


def legalize_waits(nc, max_waits=1):
    """walrus codegen caps semaphore waits per instruction; move extras onto
    NoOp instructions inserted immediately before (same engine)."""
    j = json.loads(mybir.module_to_json_bytes(nc.m))
    for fn in j.get("functions", []):
        for blk in fn.get("blocks", []):
            out = []
            for inst in blk.get("instructions", []):
                si = inst.get("sync_info") or {}
                waits = si.get("on_wait") or []
                if len(waits) > max_waits:
                    keep, extra = waits[-max_waits:], waits[:-max_waits]
                    for k, w in enumerate(extra):
                        out.append({"name": f"{inst['name']}-wsp{k}",
                                    "opcode": "NoOp", "engine": inst["engine"],
                                    "ins": [], "outs": [],
                                    "sync_info": {"on_wait": [w], "on_update": []}})
                    si = dict(si); si["on_wait"] = keep
                    inst = dict(inst); inst["sync_info"] = si
                out.append(inst)
            blk["instructions"] = out
    nc.m = mybir.module_from_json_bytes(json.dumps(j).encode())
    return nc


_NC_CACHE = {}


def _get_nc(repeat=1):
    if repeat not in _NC_CACHE:
        nc = build_nc(repeat)
        legalize_waits(nc)
        _NC_CACHE[repeat] = nc
    return _NC_CACHE[repeat]


def build_in_maps(x, lengths_x, context, lengths_ctx,
                  Wih0, Whh0, bih0, bhh0, Wih1, Whh1, bih1, bhh1, Wd, bd):
    Wt = build_weights(Wih0.astype(np.float32), Whh0.astype(np.float32),
                       bih0.astype(np.float32), bhh0.astype(np.float32),
                       Wih1.astype(np.float32), Whh1.astype(np.float32),
                       bih1.astype(np.float32), bhh1.astype(np.float32),
                       Wd.astype(np.float32), bd.astype(np.float32))
    Bn = x.shape[0] // N_CORES
    in_maps = []
    for core in range(N_CORES):
        sl = slice(core * Bn, (core + 1) * Bn)
        st = build_streams(np.ascontiguousarray(x[sl], dtype=np.float32),
                           np.asarray(lengths_x[sl], dtype=np.int64),
                           np.ascontiguousarray(context[sl], dtype=np.float32),
                           np.asarray(lengths_ctx[sl], dtype=np.int64))
        m = dict(st)
        m.pop("nmd", None)
        m.update(Wt)
        in_maps.append(m)
    return in_maps


def kernel(x, lengths_x, context, lengths_ctx,
           Wih0, Whh0, bih0, bhh0, Wih1, Whh1, bih1, bhh1, Wd, bd):
    x = np.asarray(x)
    context = np.asarray(context)
    lengths_x = np.asarray(lengths_x)
    lengths_ctx = np.asarray(lengths_ctx)
    in_maps = build_in_maps(x, lengths_x, context, lengths_ctx,
                            np.asarray(Wih0), np.asarray(Whh0), np.asarray(bih0),
                            np.asarray(bhh0), np.asarray(Wih1), np.asarray(Whh1),
                            np.asarray(bih1), np.asarray(bhh1), np.asarray(Wd),
                            np.asarray(bd))
    nc = _get_nc(1)
    res = run_bass_kernel_spmd(nc, in_maps, core_ids=list(range(N_CORES)))
    Bn = x.shape[0] // N_CORES
    outs = []
    bd32 = np.asarray(bd, dtype=np.float32)
    for core in range(N_CORES):
        sl = slice(core * Bn, (core + 1) * Bn)
        YE = res.results[core]["ye"]
        YD = res.results[core]["yd"].reshape(CD, 8, 8, Bn)
        outs.append(post_outputs(YE, YD, bd32,
                                 np.asarray(lengths_ctx[sl], dtype=np.int64), TC))
    return np.concatenate(outs, axis=0).astype(np.float32)



# revision 2
# speedup vs baseline: 1.0789x; 1.0789x over previous
"""Self-contained Trainium2 Bass kernel for nn_AutoRegressive_88837103551116.

2-layer LSTM (HID=64) over ragged sequences: warmup pass over x (per-sample
lengths), then autoregressive decode over [dense(h_top_final), context_t].
Pure data-parallel over 8 NeuronCores (batch 512 -> 64 per core).

v3 design (latency-optimized; dependent-instruction links dominate on TRN2):
  - all four gates partition-stacked by layer: PSUM mega tile [128, 4B]
    with col blocks [i | f | g | o], partitions = [l0 feats; l1 feats]
  - tanh-only activations: sigmoid(x) = (tanh(x/2)+1)/2 folded into the
    weights; ONE tanh over the whole gate tile per tick
  - doubled states h~=2h, c~=2c so every cell op is one fused
    scalar_tensor_tensor: t1=(Ti+1)*Tg, t2=(Tf+1)*c~ (gpsimd, parallel),
    c~'=(t2*0.5)+t1, th=tanh(0.5*c~'), h~=(To+1)*th
  - 4 recurrence matmuls per tick (K=128, M=128, N=64, bf16 -> FWL) +
    4 stream matmuls (lhsT [20,128]) hoisted off the critical path
  - ragged-sequence freeze: +/-BIG added to i/f pre-activations past each
    sample's length (c freezes exactly); h at the last valid step captured
    via copy_predicated masks
  - layer stagger: tick s computes l0 @ step s, l1 @ step s-1
"""
import sys

import numpy as np

try:
    import concourse.bass as bass
except ImportError:
    sys.path.insert(0, "/opt/trn_rl_repo")
    import concourse.bass as bass

import contextlib
import json

import ml_dtypes
import concourse.tile as tile
from concourse import mybir
from concourse.bass_utils import run_bass_kernel_spmd

N_CORES = 8
TW = 512
TC = 512

H = 64
IN = 16
F = 8
C = 8
BIG = 50.0

F32 = mybir.dt.float32
BF16 = mybir.dt.bfloat16
U8 = mybir.dt.uint8
AF = mybir.ActivationFunctionType
ALU = mybir.AluOpType

B = 64          # batch per core
SW = 528        # warmup ticks (need >= 513; multiple of BODY)
SD = 512        # decode ticks
CW = SW // 8    # warmup stream chunks
CD = SD // 8
BODY = 16       # ticks per For_i body
PROBE = ""      # timing-only ablations: noth|nodve|notanh|nomm


def build_weights(Wih0, Whh0, bih0, bhh0, Wih1, Whh1, bih1, bhh1, Wd, bd):
    """Stationary lhsT matrices (bf16, shared across cores).

    Scale folds: h~ = 2h (all h-inputs x0.5); sigmoid gates i,f,o computed as
    tanh(pre/2) (gate rows x0.5; o needs no extra scale since sigma(o) is the
    only use).  g rows keep scale 1 (true tanh).
    """
    b0 = (bih0 + bhh0).astype(np.float64)
    b1 = (bih1 + bhh1).astype(np.float64)
    gsl = {"i": slice(0, 64), "f": slice(64, 128), "g": slice(128, 192), "o": slice(192, 256)}
    gscale = {"i": 0.5, "f": 0.5, "g": 1.0, "o": 0.5}
    gmask = {"i": -BIG, "f": BIG, "g": 0.0, "o": 0.0}

    W = {}
    for q in gsl:
        s = gscale[q]
        # recurrence lhsT [128, 128]: rows = h~ = [h0;h1], cols = [q_l0; q_l1]
        r = np.zeros((128, 128), np.float64)
        r[0:64, 0:64] = Whh0[gsl[q]].T * s * 0.5
        r[0:64, 64:128] = Wih1[gsl[q]].T * s * 0.5
        r[64:128, 64:128] = Whh1[gsl[q]].T * s * 0.5
        W[f"r_{q}"] = r
        # warmup stream lhsT [20, 128]: rows 0:16 = x, 16 = l0 mask, 17 = l0
        # bias, 18 = l1 bias, 19 = l1 mask
        sw = np.zeros((20, 128), np.float64)
        sw[0:16, 0:64] = Wih0[gsl[q]].T * s
        sw[16, 0:64] = gmask[q]
        sw[17, 0:64] = b0[gsl[q]] * s
        sw[18, 64:128] = b1[gsl[q]] * s
        sw[19, 64:128] = gmask[q]
        W[f"sw_{q}"] = sw
        # decode stream lhsT [20, 128]: rows 0:8 = elem, 8:16 = ctx
        sd = np.zeros((20, 128), np.float64)
        sd[0:8, 0:64] = Wih0[gsl[q], 0:8].T * s
        sd[8:16, 0:64] = Wih0[gsl[q], 8:16].T * s
        sd[16, 0:64] = gmask[q]
        sd[17, 0:64] = b0[gsl[q]] * s
        sd[18, 64:128] = b1[gsl[q]] * s
        sd[19, 64:128] = gmask[q]
        W[f"sd_{q}"] = sd
    # dense: ys = Wd . h_l1 ; h~ fold -> x0.5 ; lhsT [128, 8] rows 64:128
    wd = np.zeros((128, 8), np.float64)
    wd[64:128, :] = Wd.T * 0.5
    W["wd"] = wd
    out = {k: np.ascontiguousarray(v, dtype=ml_dtypes.bfloat16) for k, v in W.items()}
    out["bd"] = np.ascontiguousarray(bd.reshape(8, 1), np.float32)
    return out


def build_streams(x, lengths_x, context, lengths_ctx):
    """Per-core streams (bf16 / u8). x [B,TW,16], context [B,TC,8]."""
    Bn = x.shape[0]
    s_idx = np.arange(SW)
    mw = (s_idx[:, None] < lengths_x[None, :]).astype(np.float32)   # [SW,B]
    mw1 = np.zeros_like(mw)
    mw1[1:] = mw[:-1]

    WA = np.zeros((SW, 20, Bn), np.float32)
    xt = np.transpose(x, (1, 2, 0))  # [TW,16,B]
    WA[0:TW, 0:16] = xt
    WA[:, 16] = 1.0 - mw
    WA[:, 17] = 1.0
    WA[:, 18] = 1.0
    WA[:, 19] = 1.0 - mw1
    NM = np.zeros((SW, 128, Bn), np.uint8)
    NM[:, 0:64] = mw[:, None, :]
    NM[:, 64:128] = mw1[:, None, :]

    md0 = np.zeros(SD, np.float32)
    md0[0:TC - 1] = 1.0
    md1 = np.ones(SD, np.float32)
    md1[0] = 0.0
    DA = np.zeros((SD, 20, Bn), np.float32)
    ctxt = np.transpose(context, (1, 2, 0))  # [TC,8,B]
    DA[0:TC - 1, 8:16] = ctxt[0:TC - 1]
    DA[:, 16] = (1.0 - md0)[:, None]
    DA[:, 17] = 1.0
    DA[:, 18] = 1.0
    DA[:, 19] = (1.0 - md1)[:, None]

    def chunked(a, nch):
        # [S, R, B] -> [nch+1, R, 8*B]  (tick-major inside chunk)
        S, R, Bn_ = a.shape
        c = a.reshape(nch, 8, R, Bn_).transpose(0, 2, 1, 3).reshape(nch, R, 8 * Bn_)
        return np.concatenate([c, np.zeros_like(c[:1])], 0)

    return dict(
        wa=np.ascontiguousarray(chunked(WA, CW), dtype=ml_dtypes.bfloat16),
        nmw=np.ascontiguousarray(chunked(NM, CW), dtype=np.uint8),
        da=np.ascontiguousarray(chunked(DA, CD), dtype=ml_dtypes.bfloat16),
    )


def post_outputs(YE, YD, bd, lengths_ctx, TC_):
    """YE [8,B], YD [CD,8,8*B] -> out [B,TC,8] with -999 padding."""
    Bn = YE.shape[1]
    out = np.zeros((Bn, TC_, F), np.float32)
    out[:, 0, :] = YE.T
    ysd = YD.reshape(CD, 8, 8, Bn).transpose(0, 2, 1, 3).reshape(SD, F, Bn)
    out[:, 1:TC_, :] = ysd[1:TC_].transpose(2, 0, 1) + bd[None, None, :]
    valid = np.arange(TC_)[None, :] < lengths_ctx[:, None]
    return np.where(valid[:, :, None], out, np.float32(-999.0))


def build_nc(repeat=1, static=False):
    nc = bass.Bass("TRN2", target_bir_lowering=False, debug=False)

    d = {}
    d["wa"] = nc.dram_tensor("wa", [CW + 1, 20, 512], BF16, kind="ExternalInput")
    d["nmw"] = nc.dram_tensor("nmw", [CW + 1, 128, 512], U8, kind="ExternalInput")
    d["da"] = nc.dram_tensor("da", [CD + 1, 20, 512], BF16, kind="ExternalInput")
    wnames = (["r_i", "r_f", "r_g", "r_o"]
              + [f"sw_{q}" for q in "ifgo"] + [f"sd_{q}" for q in "ifgo"])
    for name in wnames:
        shp = [128, 128] if name.startswith("r_") else [20, 128]
        d[name] = nc.dram_tensor(name, shp, BF16, kind="ExternalInput")
    d["wd"] = nc.dram_tensor("wd", [128, 8], BF16, kind="ExternalInput")
    d["bd"] = nc.dram_tensor("bd", [8, 1], F32, kind="ExternalInput")
    ye = nc.dram_tensor("ye", [8, B], F32, kind="ExternalOutput")
    yd = nc.dram_tensor("yd", [CD, 8, 512], F32, kind="ExternalOutput")

    with tile.TileContext(nc) as tc:
        with (
            tc.tile_pool(name="consts", bufs=1) as consts,
            tc.tile_pool(name="state", bufs=1) as state,
            tc.tile_pool(name="stream", bufs=2) as stream,
            tc.tile_pool(name="work", bufs=3) as work,
            tc.tile_pool(name="psum", bufs=2, space="PSUM") as psum,
            tc.tile_pool(name="outp", bufs=2, space="PSUM") as outp,
        ):
            W = {}
            for name in wnames + ["wd"]:
                t = consts.tile(list(d[name].shape), BF16, tag=name, name="w_" + name)
                nc.sync.dma_start(out=t, in_=d[name][:, :])
                W[name] = t
            bdt = consts.tile([8, 1], F32, tag="bd", name="w_bd")
            nc.sync.dma_start(out=bdt, in_=d["bd"][:, :])

            rb = [state.tile([128, B], BF16, tag=f"rb{i}", name=f"rb{i}") for i in range(2)]
            cc = [state.tile([128, B], F32, tag=f"cc{i}", name=f"cc{i}") for i in range(2)]
            for i in range(2):
                nc.vector.memset(rb[i], 0.0)
                nc.vector.memset(cc[i], 0.0)

            saA = stream.tile([20, 512], BF16, tag="saA")
            saB = stream.tile([20, 512], BF16, tag="saB")
            nmA = stream.tile([128, 512], U8, tag="nmA")
            nmB = stream.tile([128, 512], U8, tag="nmB")
            elem = state.tile([8, B], F32, tag="elem")
            elemb = state.tile([8, B], BF16, tag="elemb")
            hkeep = state.tile([128, B], BF16, tag="hkeep")
            nc.vector.memset(hkeep, 0.0)

            def tick(sl, sa, nm, decode, ops=None):
                par = sl % 2
                rbp, rbn = rb[par], rb[1 - par]
                ccp, ccn = cc[par], cc[1 - par]
                t8 = sl % 8
                colB = slice(t8 * B, (t8 + 1) * B)
                mega = psum.tile([128, 4 * B], F32, tag="mega", name="mega")

                pre = "sd_" if decode else "sw_"
                # stream matmuls (start) then recurrence matmuls (stop)
                for qi, q in enumerate("ifgo"):
                    nc.tensor.matmul(mega[:, qi * B:(qi + 1) * B], W[pre + q],
                                     sa[0:20, colB], start=True, stop=False,
                                     skip_group_check=True)
                for qi, q in enumerate("ifgo"):
                    nc.tensor.matmul(mega[:, qi * B:(qi + 1) * B], W["r_" + q],
                                     rbp, start=False, stop=True,
                                     skip_group_check=True)

                # one tanh over all gates
                T = work.tile([128, 4 * B], F32, tag="T", name="T")
                if PROBE == "notanh":
                    nc.vector.tensor_copy(T, mega[:, :])
                else:
                    nc.scalar.activation(T, mega[:, :], AF.Tanh)
                Ti, Tf = T[:, 0:B], T[:, B:2 * B]
                Tg, To = T[:, 2 * B:3 * B], T[:, 3 * B:4 * B]

                # cell (doubled states): c~' = ((Tf+1)*c~)*0.5 + (Ti+1)*Tg
                t1 = work.tile([128, B], F32, tag="t1", name="t1")
                t2 = work.tile([128, B], F32, tag="t2", name="t2")
                th = work.tile([128, B], F32, tag="th", name="th")
                if PROBE == "nodve":
                    nc.vector.scalar_tensor_tensor(ccn, Tf, 1.0, ccp, ALU.add, ALU.mult)
                else:
                    nc.vector.scalar_tensor_tensor(t2, Tf, 1.0, ccp, ALU.add, ALU.mult)
                    nc.vector.scalar_tensor_tensor(t1, Ti, 1.0, Tg, ALU.add, ALU.mult)
                    nc.vector.scalar_tensor_tensor(ccn, t2, 0.5, t1, ALU.mult, ALU.add)
                if PROBE == "noth":
                    nc.vector.scalar_tensor_tensor(rbn, To, 1.0, ccn, ALU.add, ALU.mult)
                else:
                    nc.scalar.activation(th, ccn, AF.Tanh, scale=0.5)
                    nc.vector.scalar_tensor_tensor(rbn, To, 1.0, th, ALU.add, ALU.mult)

                if nm is not None:
                    nc.vector.copy_predicated(hkeep, nm[:, colB], rbn)
                if ops is not None:
                    nc.tensor.matmul(ops[:, colB], W["wd"], rbn, start=True, stop=True)
                return rbn

            rep_cm = tc.For_i(0, repeat, 1) if repeat > 1 else contextlib.nullcontext()
            rep_cm.__enter__()

            # ================= warmup =================
            for i in range(2):
                if repeat > 1:
                    nc.vector.memset(rb[i], 0.0)
                    nc.vector.memset(cc[i], 0.0)
            if repeat > 1:
                nc.vector.memset(hkeep, 0.0)
            NCHB = BODY // 8  # chunks per body (even, for buffer parity)
            sa2 = [saA, saB]
            nm2 = [nmA, nmB]
            nbody_w = SW // BODY
            nc.sync.dma_start(out=saA, in_=d["wa"][0, :, :])
            nc.sync.dma_start(out=nmA, in_=d["nmw"][0, :, :])

            def warm_body(j, idx):
                for c in range(NCHB):
                    nxt = idx(c + 1)
                    nc.sync.dma_start(out=sa2[(c + 1) % 2], in_=d["wa"][nxt, :, :])
                    nc.sync.dma_start(out=nm2[(c + 1) % 2], in_=d["nmw"][nxt, :, :])
                    for t in range(8):
                        tick(c * 8 + t, sa2[c % 2], nm2[c % 2], False)

            if static:
                for j in range(nbody_w):
                    warm_body(j, lambda c, j=j: j * NCHB + c)
            else:
                warm_body(0, lambda c: c)
                with tc.For_i(1, nbody_w, 1, hint_engines=(mybir.EngineType.PE,)) as j:
                    idxs = [nc.snap(j * NCHB + c) for c in range(NCHB + 1)]
                    warm_body(j, lambda c: idxs[c])

            # ================= elem =================
            nc.vector.tensor_copy(rb[0], hkeep)
            pe = outp.tile([8, 512], F32, tag="ops", name="pe")
            nc.tensor.matmul(pe[:, 0:B], W["wd"], hkeep, start=True, stop=True)
            nc.scalar.activation(elem, pe[:, 0:B], AF.Identity, bias=bdt[:, 0:1])
            nc.vector.tensor_copy(elemb, elem)
            nc.sync.dma_start(out=ye[:, :], in_=elem)

            # ================= decode =================
            nbody_d = SD // BODY
            nc.sync.dma_start(out=saA, in_=d["da"][0, :, :])

            def put_elem(sa):
                # broadcast elem into stream rows 0:8 (8 ticks x B cols)
                o = sa[0:8, :].rearrange("p (t c) -> p t c", t=8)
                i = elemb[0:8, 0:B].unsqueeze(1).broadcast_to((8, 8, B))
                nc.vector.tensor_copy(o, i)

            put_elem(saA)

            def dec_body(j, idx, first=False):
                for c in range(NCHB):
                    nxt = idx(c + 1)
                    nc.sync.dma_start(out=sa2[(c + 1) % 2], in_=d["da"][nxt, :, :])
                    put_elem(sa2[(c + 1) % 2])
                    ops = outp.tile([8, 512], F32, tag="ops", name=f"ops{c}")
                    for t in range(8):
                        tick(c * 8 + t, sa2[c % 2], None, True, ops=ops)
                        if first and c == 0 and t == 0:
                            nc.vector.tensor_copy(rb[1][64:128, :], rb[0][64:128, :])
                    oso = work.tile([8, 512], F32, tag="oso", name=f"oso{c}")
                    nc.vector.tensor_copy(oso, ops)
                    nc.sync.dma_start(out=yd[idx(c), :, :], in_=oso)

            if static:
                for j in range(nbody_d):
                    dec_body(j, lambda c, j=j: j * NCHB + c, first=(j == 0))
            else:
                dec_body(0, lambda c: c, first=True)
                with tc.For_i(1, nbody_d, 1, hint_engines=(mybir.EngineType.PE,)) as j:
                    idxs = [nc.snap(j * NCHB + c) for c in range(NCHB + 1)]
                    dec_body(j, lambda c: idxs[c])

            rep_cm.__exit__(None, None, None)

    return nc


def legalize_waits(nc, max_waits=1):
    """walrus codegen caps semaphore waits per instruction; move extras onto
    NoOp instructions inserted immediately before (same engine)."""
    j = json.loads(mybir.module_to_json_bytes(nc.m))
    for fn in j.get("functions", []):
        for blk in fn.get("blocks", []):
            out = []
            for inst in blk.get("instructions", []):
                si = inst.get("sync_info") or {}
                waits = si.get("on_wait") or []
                if len(waits) > max_waits:
                    keep, extra = waits[-max_waits:], waits[:-max_waits]
                    for k, w in enumerate(extra):
                        out.append({"name": f"{inst['name']}-wsp{k}",
                                    "opcode": "NoOp", "engine": inst["engine"],
                                    "ins": [], "outs": [],
                                    "sync_info": {"on_wait": [w], "on_update": []}})
                    si = dict(si); si["on_wait"] = keep
                    inst = dict(inst); inst["sync_info"] = si
                out.append(inst)
            blk["instructions"] = out
    nc.m = mybir.module_from_json_bytes(json.dumps(j).encode())
    return nc


_NC_CACHE = {}


STATIC = False


def _get_nc(repeat=1):
    if repeat not in _NC_CACHE:
        nc = build_nc(repeat, static=STATIC)
        legalize_waits(nc)
        _NC_CACHE[repeat] = nc
    return _NC_CACHE[repeat]


def build_in_maps(x, lengths_x, context, lengths_ctx,
                  Wih0, Whh0, bih0, bhh0, Wih1, Whh1, bih1, bhh1, Wd, bd):
    Wt = build_weights(np.asarray(Wih0, np.float64), np.asarray(Whh0, np.float64),
                       np.asarray(bih0, np.float64), np.asarray(bhh0, np.float64),
                       np.asarray(Wih1, np.float64), np.asarray(Whh1, np.float64),
                       np.asarray(bih1, np.float64), np.asarray(bhh1, np.float64),
                       np.asarray(Wd, np.float64), np.asarray(bd, np.float64))
    Bn = x.shape[0] // N_CORES
    in_maps = []
    for core in range(N_CORES):
        sl = slice(core * Bn, (core + 1) * Bn)
        st = build_streams(np.ascontiguousarray(x[sl], dtype=np.float32),
                           np.asarray(lengths_x[sl], dtype=np.int64),
                           np.ascontiguousarray(context[sl], dtype=np.float32),
                           np.asarray(lengths_ctx[sl], dtype=np.int64))
        m = dict(st)
        m.update(Wt)
        in_maps.append(m)
    return in_maps


def kernel(x, lengths_x, context, lengths_ctx,
           Wih0, Whh0, bih0, bhh0, Wih1, Whh1, bih1, bhh1, Wd, bd):
    x = np.asarray(x)
    context = np.asarray(context)
    lengths_x = np.asarray(lengths_x)
    lengths_ctx = np.asarray(lengths_ctx)
    in_maps = build_in_maps(x, lengths_x, context, lengths_ctx,
                            np.asarray(Wih0), np.asarray(Whh0), np.asarray(bih0),
                            np.asarray(bhh0), np.asarray(Wih1), np.asarray(Whh1),
                            np.asarray(bih1), np.asarray(bhh1), np.asarray(Wd),
                            np.asarray(bd))
    nc = _get_nc(1)
    res = run_bass_kernel_spmd(nc, in_maps, core_ids=list(range(N_CORES)))
    Bn = x.shape[0] // N_CORES
    outs = []
    bd32 = np.asarray(bd, dtype=np.float32)
    for core in range(N_CORES):
        sl = slice(core * Bn, (core + 1) * Bn)
        YE = res.results[core]["ye"]
        YD = res.results[core]["yd"]
        outs.append(post_outputs(YE, YD, bd32,
                                 np.asarray(lengths_ctx[sl], dtype=np.int64), TC))
    return np.concatenate(outs, axis=0).astype(np.float32)
